# revision 43
# baseline (speedup 1.0000x reference)
"""DeepSeekV2-style single-token decode attention on 8 Trainium2 NeuronCores.

Strategy (all host-side prep is layout/sharding only; all FLOPs run on device):
  - Batch-shard attention: 8 sequences per core. Sequences are sorted by
    length and grouped so the per-slot static chunk budgets (baked into the
    single SPMD program) hug the actual seq_lens.
  - Tensor-parallel projections: w_qkv^T column-slice per core, with an
    AllToAll to reshard q per-sequence; w_o is split by OUTPUT columns, fed
    by AllGathers of the (tiny) attention results, so each core writes its
    own disjoint [64, 640] slice of the output — no ReduceScatter needed.
  - Slots are processed in two groups (ending on the tiniest slot); the
    first group's attn AllGather and its w_o matmuls hide under the KV
    stream, so only the second group's AllGather + matmul trail it.
  - Everything that moves over HBM is bf16 (weights, activations, KV cache,
    probs, attn); accumulation stays f32 in PSUM.
  - K cache plane is host-transposed per sequence to [d, s]; V plane is
    host-interleaved to [p, chunk, d+1] so both stream as large contiguous
    per-partition DMA lines. The ones-column on V accumulates the softmax
    denominator in the same PSUM tile as P@V.
  - Softmax masking is fused into the ACT exp via the per-partition bias
    operand (scores kept in [s, h] layout), using host-built mask columns.
"""

import ml_dtypes
import numpy as np

import concourse.bass as bass
import concourse.tile as tile
from concourse import bacc, mybir
from concourse.bass_utils import run_bass_kernel_spmd

F32 = mybir.dt.float32
BF16 = mybir.dt.bfloat16
EXP = mybir.ActivationFunctionType.Exp

H, D = 32, 128
HID = 5120
Q_SIZE = H * D            # 4096
KV_SIZE = 576
QKV_OUT = Q_SIZE + KV_SIZE  # 4672
QKV_USED = Q_SIZE + D     # 4224: only kv[:, :D] is consumed downstream
B, S_MAX = 64, 8192
SCALE = float(D) ** -0.5
NC = 8                    # cores
BPC = B // NC             # 8 seqs per core
OSL = QKV_USED // NC      # 528 qkv output slice per core
WSL = HID // NC           # 640 w_o output-column slice per core
NEG = -30000.0
CH_PIECE = 32             # kv chunks (of 128 positions) per DMA piece
GROUP = 4                 # score chunks per psum/exp group
HC = HID // 128           # 40 hidden chunks
KC = Q_SIZE // 128        # 32 w_o contraction chunks

# slot processing order: start small (attention ramps while q is in flight),
# end with the tiniest slot so the last attn lags the stream minimally; the
# first group's AllGather hides under the kv stream, only the second trails
SLOT_ORDER = [6, 5, 4, 3, 2, 1, 0, 7]
AG_GROUPS = [(6, 5, 4, 3, 2), (1, 0, 7)]
NG = len(AG_GROUPS)
GSEQS = [len(g) * NC for g in AG_GROUPS]   # sequences per AllGather group

_BUILD_CACHE = {}
PROFILE = False       # set True to capture an NTFF trace on the next kernel() call
LAST_RES = None       # BassKernelResults of the last run (for test harnesses)
DEBUG = False         # add intermediate-dump outputs to the build
REPEAT = 1            # run the whole body N times inside one NEFF (timing)

OCS = [128, 128, 128, 128, OSL - 512]   # qkv slice partition chunks


def _build(budgets, nontriv_key):
    """Build + compile the single SPMD program for the given per-slot chunk
    budgets and per-(slot, group) non-trivial-mask flags."""
    nontriv = set(nontriv_key)
    nc = bacc.Bacc("TRN2", target_bir_lowering=False, debug=False, num_devices=NC)

    hT = nc.dram_tensor("hT", [128, HC, B], BF16, kind="ExternalInput").ap()
    wqs = [nc.dram_tensor(f"wq{oc}", [128, HC, OCS[oc]], BF16,
                          kind="ExternalInput").ap() for oc in range(5)]
    wo = nc.dram_tensor("wo", [128, KC, WSL], BF16, kind="ExternalInput").ap()
    ntb = nc.dram_tensor("ntb", [1, BPC], F32, kind="ExternalInput").ap()
    kts, vgs = [], []
    for j in range(BPC):
        s_j = budgets[j] * 128
        kts.append(nc.dram_tensor(f"kt{j}", [D, s_j], BF16, kind="ExternalInput").ap())
        vgs.append(nc.dram_tensor(
            f"vg{j}", [128, budgets[j], D + 1], BF16, kind="ExternalInput").ap())
    TOTB = sum(budgets)
    moff = [sum(budgets[:j]) for j in range(BPC)]
    mka = nc.dram_tensor("mka", [128, TOTB], F32, kind="ExternalInput").ap()
    outp = nc.dram_tensor("outp", [B, WSL], F32, kind="ExternalOutput").ap()

    a1_in = nc.dram_tensor("a1_in", [QKV_USED, BPC], BF16).ap()
    a1_out = nc.dram_tensor("a1_out", [QKV_USED, BPC], BF16).ap()
    at_ins = [nc.dram_tensor(f"at_in{g}", [len(AG_GROUPS[g]), Q_SIZE], BF16).ap()
              for g in range(NG)]
    ag_outs = [nc.dram_tensor(f"ag_out{g}", [GSEQS[g], Q_SIZE], BF16).ap()
               for g in range(NG)]

    groups = [list(range(NC))]
    WO_KK = KC // 4   # w_o load chunk (in q-chunks), matching the mm kc order

    with tile.TileContext(nc) as tc:
        with (
            tc.tile_pool(name="wts", bufs=2) as wts_pool,
            tc.tile_pool(name="acts", bufs=1) as acts_pool,
            tc.tile_pool(name="kv", bufs=4) as kv_pool,
            tc.tile_pool(name="probs", bufs=4) as probs_pool,
            tc.tile_pool(name="small", bufs=2) as small_pool,
            tc.tile_pool(name="ao", bufs=2) as ao_pool,
            tc.tile_pool(name="psA", bufs=2, space="PSUM") as psA,
            tc.tile_pool(name="psSc", bufs=3, space="PSUM") as psSc,
            tc.tile_pool(name="psAt", bufs=2, space="PSUM") as psAt,
        ):
          def body():
            # ---------- Phase 1: qkv projection (TP column slice) ----------
            hT_t = acts_pool.tile([128, HC, B], BF16)
            nc.sync.dma_start(hT_t[:, :, :], hT[:, :, :])

            a1_view = a1_in.rearrange("(d o) j -> o d j", d=NC)  # [OSL, NC, BPC]
            for oc in range(5):
                osz = OCS[oc]
                wq_t = wts_pool.tile([128, HC, 128], BF16, tag="wq")
                nc.sync.dma_start(wq_t[:, :, :osz], wqs[oc][:, :, :])
                ps_q = psA.tile([128, B], F32, tag="mm")
                for hc in range(HC):
                    nc.tensor.matmul(
                        ps_q[:osz, :],
                        lhsT=wq_t[:, hc, :osz],
                        rhs=hT_t[:, hc, :],
                        start=(hc == 0), stop=(hc == HC - 1),
                    )
                q_sb = small_pool.tile([128, B], BF16, tag="qsb")
                nc.vector.tensor_copy(q_sb[:osz, :], ps_q[:osz, :])
                nc.sync.dma_start(
                    a1_view[oc * 128:oc * 128 + osz, :, :],
                    q_sb[:osz, :].rearrange("o (d j) -> o d j", d=NC),
                )

            nc.gpsimd.collective_compute(
                "AllToAll", mybir.AluOpType.bypass, replica_groups=groups,
                ins=[a1_in[:, :]], outs=[a1_out[:, :]],
            )

            # ---------- Phase 2: per-core q / kv_new ----------
            qt_t = acts_pool.tile([128, H, BPC], BF16)   # q^T: [d, head, slot]
            nc.sync.dma_start(
                qt_t[:, :, :],
                a1_out[0:Q_SIZE, :].rearrange("(h p) j -> p h j", p=128),
            )
            kvnT_t = acts_pool.tile([128, BPC], BF16)    # kv_new^T: [d, slot]
            nc.sync.dma_start(kvnT_t[:, :], a1_out[Q_SIZE:Q_SIZE + D, :])
            # transpose kv_new to rows on a single partition (+ ones columns)
            ident = acts_pool.tile([128, 128], BF16, tag="id")
            make_identity(nc, ident[:, :])
            ps_kn = psSc.tile([BPC, D], BF16, tag="sc")
            nc.tensor.transpose(ps_kn[:, :], kvnT_t[:, :], ident[:, :])
            kn8_t = small_pool.tile([BPC, D], BF16, tag="kn8")
            nc.vector.tensor_copy(kn8_t[:, :], ps_kn[:, :])
            kvnr_t = acts_pool.tile([1, BPC * (D + 1)], BF16)  # kv_new rows + ones
            nc.vector.memset(kvnr_t[:, :], 1.0)
            nc.sync.dma_start(
                kvnr_t[0:1, :].rearrange("p (j e) -> p j e", e=D + 1)[:, :, 0:D],
                kn8_t[:, :])
            ntb_t = acts_pool.tile([1, BPC], F32)
            nc.sync.dma_start(ntb_t[:, :], ntb[:, :])
            mka_t = acts_pool.tile([128, TOTB], F32, tag="mka")
            nc.sync.dma_start(mka_t[:, :], mka[:, :])

            at_all_t = acts_pool.tile([H, BPC, D], BF16)  # attn for all 8 slots
            wo_t = acts_pool.tile([128, KC, WSL], BF16, tag="wo")
            o_sb = acts_pool.tile([B, WSL], F32, tag="osb")

            def ag_group(g):
                nc.gpsimd.collective_compute(
                    "AllGather", mybir.AluOpType.bypass, replica_groups=groups,
                    ins=[at_ins[g][:, :]], outs=[ag_outs[g][:, :]],
                )

            ao_ts = {}

            def phase4_transposes(g):
                ns = GSEQS[g]
                ao_t = ao_pool.tile([128, KC, ns], BF16, tag=f"ao{g}")
                ao_ts[g] = ao_t
                for kc in range(KC):
                    nc.sync.dma_start(
                        ao_t[:, kc, :], ag_outs[g][:, kc * 128:(kc + 1) * 128],
                        transpose=True)

            def phase4_mms(g, row0):
                """Project group g's gathered attn through this core's w_o
                column slice and write outp rows [row0, row0+GSEQS[g])."""
                ns = GSEQS[g]
                ao_t = ao_ts[g]
                og_sb = small_pool.tile([ns, WSL], F32, tag=f"og{g}")
                for w0 in range(0, WSL, 320):
                    ps_o = psA.tile([ns, 320], F32, tag="mm")
                    for kc in range(KC):
                        nc.tensor.matmul(
                            ps_o[:, :], lhsT=ao_t[:, kc, :],
                            rhs=wo_t[:, kc, w0:w0 + 320],
                            start=(kc == 0), stop=(kc == KC - 1),
                        )
                    nc.vector.tensor_copy(og_sb[:, w0:w0 + 320], ps_o[:, :])
                nc.sync.dma_start(outp[row0:row0 + ns, :], og_sb[:, :])

            # ---------- Phase 3: attention, shortest slots first ----------
            for si, j in enumerate(SLOT_ORDER):
                bj = budgets[j]
                mof = moff[j]
                attn_ps = psAt.tile([H, D + 1], F32, tag="at")
                qt_j = qt_t[:, :, j]
                n_mm = 0
                # piece sizes: cap the final piece of the last two slots at 8
                # chunks so the tail attention drains fast after the stream
                pieces = []
                rem = bj
                while rem > 0:
                    pc_ = min(CH_PIECE, rem)
                    if (si >= BPC - 2 and pc_ == rem and 8 < rem <= CH_PIECE
                            and (rem - 8) % GROUP == 0):
                        pieces.extend([rem - 8, 8])
                        rem = 0
                    elif si >= BPC - 2 and pc_ == rem and 4 < rem <= 12:
                        pieces.extend([4, rem - 4])
                        rem = 0
                    else:
                        pieces.append(pc_)
                        rem -= pc_
                p0 = 0
                for pc in pieces:
                    kt_t = kv_pool.tile([128, CH_PIECE * 128], BF16, tag="kt")
                    nc.sync.dma_start(
                        kt_t[:, :pc * 128], kts[j][:, p0 * 128:(p0 + pc) * 128])
                    vg_t = kv_pool.tile([128, CH_PIECE, D + 1], BF16, tag="vg")
                    nc.sync.dma_start(vg_t[:, :pc, :], vgs[j][:, p0:p0 + pc, :])
                    for g0 in range(0, pc, GROUP):
                        gs = min(GROUP, pc - g0)
                        ps_sc = psSc.tile([128, GROUP * H], F32, tag="sc")
                        for k in range(gs):
                            nc.tensor.matmul(
                                ps_sc[:, k * H:(k + 1) * H],
                                lhsT=kt_t[:, (g0 + k) * 128:(g0 + k + 1) * 128],
                                rhs=qt_j,
                                start=True, stop=True,
                            )
                        pt = probs_pool.tile([128, GROUP * H], BF16, tag="pt")
                        grp_id = (p0 + g0) // GROUP
                        if (j, grp_id) in nontriv:
                            for k in range(gs):
                                ch = p0 + g0 + k
                                nc.scalar.activation(
                                    pt[:, k * H:(k + 1) * H],
                                    ps_sc[:, k * H:(k + 1) * H],
                                    EXP, bias=mka_t[:, mof + ch:mof + ch + 1],
                                    scale=SCALE,
                                )
                        else:
                            nc.scalar.activation(
                                pt[:, :gs * H], ps_sc[:, :gs * H], EXP, scale=SCALE)
                        for k in range(gs):
                            nc.tensor.matmul(
                                attn_ps[:, :],
                                lhsT=pt[:, k * H:(k + 1) * H],
                                rhs=vg_t[:, g0 + k, :],
                                start=(n_mm == 0), stop=False,
                            )
                            n_mm += 1
                    p0 += pc
                # new-token term (Kc=1 outer product adds p_new to attn and denom)
                ps_nt = psSc.tile([1, H], F32, tag="sc")
                nc.tensor.matmul(
                    ps_nt[:, :], lhsT=kvnT_t[:, j:j + 1], rhs=qt_j,
                    start=True, stop=True)
                pn_t = small_pool.tile([1, H], BF16, tag="pn")
                nc.scalar.activation(
                    pn_t[:, :], ps_nt[:, :], EXP,
                    bias=ntb_t[0:1, j:j + 1], scale=SCALE)
                nc.tensor.matmul(
                    attn_ps[:, :], lhsT=pn_t[0:1, :],
                    rhs=kvnr_t[0:1, j * (D + 1):(j + 1) * (D + 1)],
                    start=False, stop=True)

                rc_t = small_pool.tile([H, 1], F32, tag="rc")
                nc.vector.reciprocal(rc_t[:, :], attn_ps[:, D:D + 1])
                nc.vector.tensor_scalar_mul(
                    at_all_t[:, si, :], attn_ps[:, 0:D], rc_t[:, :])
                # at group end, stage attn rows: at_in{g}[jj, (h x)] <- at_all[h, x, si]
                gend = 0
                for g in range(NG):
                    gend += len(AG_GROUPS[g])
                    if si == gend - 1:
                        g0 = gend - len(AG_GROUPS[g])
                        nc.sync.dma_start(
                            at_ins[g][:, :].rearrange(
                                "j (h x) -> h j x", x=128),
                            at_all_t[:, g0:gend, :])
                        ag_group(g)   # collective only — no FIFO hazard
                        break

            # ---------- Phase 4: after the stream in program order (so the
            # scheduler never slots collective-gated work into the stream).
            # The w_o load runs in the SP-idle window under the trailing
            # AllGather; group-0 matmuls consume its chunks as they land.
            phase4_transposes(0)
            for i in range(4):
                nc.sync.dma_start(
                    wo_t[:, i * WO_KK:(i + 1) * WO_KK, :],
                    wo[:, i * WO_KK:(i + 1) * WO_KK, :])
            row0 = 0
            for g in range(NG):
                if g > 0:
                    phase4_transposes(g)
                phase4_mms(g, row0)
                row0 += GSEQS[g]

          for _rep in range(REPEAT):
              body()
          if DEBUG:
              for nm, s_ap in [("dbg_a1", a1_out), ("dbg_ag0", ag_outs[0])]:
                  dst = nc.dram_tensor(nm, list(s_ap.shape), s_ap.dtype,
                                       kind="ExternalOutput").ap()
                  nc.sync.dma_start(dst[:, :], s_ap[:, :])

    nc.compile()
    return nc


def _prepare(hidden_states, positions, kv_cache, slot_mapping, seq_lens, w_qkv, w_o):
    """Host-side sharding/layout prep. Returns (nc, in_maps, col_seq)."""
    hidden_states = np.asarray(hidden_states, dtype=np.float32)
    kv_cache = np.asarray(kv_cache, dtype=np.float32)
    slot_mapping = np.asarray(slot_mapping)
    seq_lens = np.asarray(seq_lens)
    w_qkv = np.asarray(w_qkv, dtype=np.float32)
    w_o = np.asarray(w_o, dtype=np.float32)

    sl = seq_lens.astype(np.int64)
    sm = slot_mapping.astype(np.int64)

    # sort by length desc; core c slot j <- rank 8j + c (consecutive ranks
    # share a column so the column max hugs each member's length)
    order = np.argsort(-sl, kind="stable")
    seq_of = np.empty((NC, BPC), dtype=np.int64)
    for j in range(BPC):
        for c in range(NC):
            seq_of[c, j] = order[NC * j + c]

    # outp row r = row0(g) + d*len(group g) + jj maps to seq_of[d, AG_GROUPS[g][jj]]
    col_seq = np.empty(B, dtype=np.int64)
    r = 0
    for g in range(NG):
        for d in range(NC):
            for jj in range(len(AG_GROUPS[g])):
                col_seq[r] = seq_of[d, AG_GROUPS[g][jj]]
                r += 1

    budgets = []
    for j in range(BPC):
        max_l = int(sl[seq_of[:, j]].max())
        budgets.append(max(1, -(-max_l // 128)))
    budgets = tuple(budgets)

    # masks + non-trivial group flags
    masks = [np.zeros((NC, 128, budgets[j]), dtype=np.float32) for j in range(BPC)]
    nontriv = set()
    for c in range(NC):
        for j in range(BPC):
            b = seq_of[c, j]
            L, slot = int(sl[b]), int(sm[b])
            n = budgets[j] * 128
            m = np.zeros(n, dtype=np.float32)
            if L < n:
                m[L:] = NEG
            if slot < n:
                m[slot] = NEG
            mc = m.reshape(budgets[j], 128)
            masks[j][c] = mc.T
            for ch in np.nonzero(mc.any(axis=1))[0]:
                nontriv.add((j, int(ch) // GROUP))
    nontriv_key = tuple(sorted(nontriv))

    key = (budgets, nontriv_key, DEBUG, REPEAT)
    if key not in _BUILD_CACHE:
        _BUILD_CACHE[key] = _build(budgets, nontriv_key)
    nc = _BUILD_CACHE[key]

    bf = ml_dtypes.bfloat16
    # hT columns in (d, j) order to match a1's AllToAll block structure
    hT = np.ascontiguousarray(hidden_states[seq_of.reshape(-1), 0, :].T)
    hT_il = np.ascontiguousarray(
        hT.reshape(HC, 128, B).transpose(1, 0, 2)).astype(bf)       # [128, HC, B]
    wqT_il = w_qkv[:QKV_USED].T.reshape(HC, 128, QKV_USED)\
        .transpose(1, 0, 2)                                          # [128, HC, 4224]
    woT = w_o.T                                                     # [4096, HID]

    in_maps = []
    for c in range(NC):
        m = {
            "hT": hT_il,
            "wo": np.ascontiguousarray(
                woT[:, c * WSL:(c + 1) * WSL].reshape(KC, 128, WSL)
                .transpose(1, 0, 2)).astype(bf),
            "ntb": np.where(sm[seq_of[c]] < sl[seq_of[c]], 0.0, NEG
                            ).astype(np.float32).reshape(1, BPC),
        }
        off = c * OSL
        for oc in range(5):
            osz = OCS[oc]
            m[f"wq{oc}"] = np.ascontiguousarray(
                wqT_il[:, :, off + oc * 128:off + oc * 128 + osz]).astype(bf)
        for j in range(BPC):
            b = seq_of[c, j]
            n = budgets[j] * 128
            m[f"kt{j}"] = np.ascontiguousarray(kv_cache[0, b, :n, :].T).astype(bf)
            vg = np.empty((n, D + 1), dtype=np.float32)
            vg[:, :D] = kv_cache[1, b, :n, :]
            vg[:, D] = 1.0
            m[f"vg{j}"] = np.ascontiguousarray(
                vg.reshape(budgets[j], 128, D + 1).transpose(1, 0, 2)).astype(bf)
        m["mka"] = np.ascontiguousarray(
            np.concatenate([masks[j][c] for j in range(BPC)], axis=1))
        in_maps.append(m)

    return nc, in_maps, col_seq


def kernel(hidden_states, positions, kv_cache, slot_mapping, seq_lens, w_qkv, w_o):
    nc, in_maps, col_seq = _prepare(
        hidden_states, positions, kv_cache, slot_mapping, seq_lens, w_qkv, w_o)
    res = run_bass_kernel_spmd(nc, in_maps, list(range(NC)), trace=PROFILE)
    global LAST_RES
    LAST_RES = res

    out = np.empty((B, 1, HID), dtype=np.float32)
    for c in range(NC):
        shard = res.results[c]["outp"]     # [B, WSL]: all seqs, my columns
        out[col_seq, 0, c * WSL:(c + 1) * WSL] = shard
    return out
